# revision 12
# baseline (speedup 1.0000x reference)
"""Trainium2 Bass kernel for nn_AttentionHead (B=4, S=4096, H=1024, D=64).

Reference computation (note the unusual K-first ordering):
    K = x @ Wk.T; Q = x @ Wq.T; V = x @ Wv.T            [B,S,D]
    scores[b,i,j] = (K[b,i] . Q[b,j]) / sqrt(D)         [B,S,S]
    scores[:, :, j] = -1e12 where mask[:, j] == 0
    out = softmax(scores, axis=2) @ V                   [B,S,D]

Sharding: 8 cores = 4 batches x 2 key-row chunks of 2048. Each core gets a
batch's x ROLLED so its own key rows are always rows [0, 2048) — the SPMD
program is identical across cores. Softmax runs over the full (rolled) query
axis on every core, so rolling is correctness-neutral.

Per-core pipeline (bf16 matmuls, fp32 accumulation), software-pipelined so PE
chases the DMA stream with dense work (keeps the HAM clock-gate at 8/8):
  - x is DMA-cast to bf16 (DRAM->DRAM, SWDGE) per s-quarter and
    xbar-DMA-transposed into SBUF (sync HWDGE queue only — running the xbar
    on both HWDGE queues corrupts data).
  - One [Wq|Wv] stationary gives Q^T (rows 0:64) and V^T (rows 64:128) per
    512-col block; K^T separately over own 2048 rows; V^T -> V via PE
    transposes (identity matmul). V gets a ones column (softmax denominator).
  - Query-tile loop t=0..31, interleaved in emission order with the
    later-quarter projections: scores^T = Q^T_t.T @ K^T on PE (two 1024-wide
    i-halves); exp(0.125*s + maskbias[j]) on ACT with the mask folded into
    the per-partition bias (masked queries underflow to exactly 0); PE
    accumulates V'_t.T @ P^T_t into out'^T [65, 1024] per i-half — rows 0:64
    numerator^T, row 64 denominator, one PSUM bank per 512-col window so
    accumulation groups never interleave within a bank.
  - Finale: PE-transpose out'^T via identity matmul, then
    out = numerator * reciprocal(denominator) on DVE; one DMA store.
"""

import numpy as np

B, S, H, D = 4, 4096, 1024, 64
N_CORES = 8
SC = S // 2  # key rows per core
HC = H // 128  # contraction chunks
JT = S // 128  # query tiles
NEG = -30000.0
N_WARM = 64

_CACHE = {}


def _build():
    import concourse.bass as bass
    import concourse.tile as tile
    from concourse import bacc, mybir

    dt = mybir.dt
    AF = mybir.ActivationFunctionType

    nc = bacc.Bacc(
        "TRN2", target_bir_lowering=False, debug=False, num_devices=N_CORES
    )
    x = nc.dram_tensor("x", [S, H], dt.float32, kind="ExternalInput").ap()
    wqv = nc.dram_tensor("wqv", [H, 2 * D], dt.float32, kind="ExternalInput").ap()
    wkt = nc.dram_tensor("wkt", [H, D], dt.float32, kind="ExternalInput").ap()
    mb = nc.dram_tensor("mb", [128, JT], dt.float32, kind="ExternalInput").ap()
    ident = nc.dram_tensor("ident", [128, 128], dt.float32, kind="ExternalInput").ap()
    out = nc.dram_tensor("out", [SC, D], dt.float32, kind="ExternalOutput").ap()
    xbf = nc.dram_tensor("xbf", [S, H], dt.bfloat16).ap()

    with (
        tile.TileContext(nc) as tc,
        tc.tile_pool(name="persist", bufs=1) as persist,
        tc.tile_pool(name="ptile", bufs=4) as ptile,
        tc.tile_pool(name="accs", bufs=2) as accs,
        tc.tile_pool(name="fin", bufs=2) as fin,
    ):
        qt = persist.tile([128, S], dt.bfloat16)  # rows 0:64 = Q^T
        kt = persist.tile([128, SC], dt.bfloat16)  # rows 0:64 = K^T
        vtsb = persist.tile([128, S], dt.bfloat16)  # rows 64:128 = V^T
        vp = persist.tile([128, JT, D + 1], dt.bfloat16)
        mb_sb = persist.tile([128, JT], dt.float32)
        id_f32 = persist.tile([128, 128], dt.float32)
        id_bf = persist.tile([128, 128], dt.bfloat16)
        wtile = persist.tile([128, 512], dt.bfloat16)
        xT = persist.tile([128, HC, S], dt.bfloat16)
        wqv_sb = persist.tile([128, HC, 2 * D], dt.bfloat16)
        wk_sb = persist.tile([128, HC, D], dt.bfloat16)

        nc.sync.dma_start(mb_sb[:], mb[:])
        nc.sync.dma_start(id_f32[:], ident[:])
        nc.gpsimd.dma_start(id_bf[:], ident[:])
        nc.gpsimd.dma_start(wqv_sb[:], wqv.rearrange("(c p) d -> p c d", p=128))
        nc.gpsimd.dma_start(wk_sb[:], wkt.rearrange("(c p) d -> p c d", p=128))
        nc.vector.memset(vp[:, :, D], 1.0)
        nc.vector.memset(wtile[:], 0.0)

        # --- x^T DMA chain: cast (gpsimd) then xbar transpose (sync only) ---
        # First quarter is split in two for a faster pipeline ramp.
        def emit_xt_chain():
            for r0, r1 in [(0, 512), (512, 1024)]:
                nc.gpsimd.dma_start(xbf[r0:r1, :], x[r0:r1, :])
                for hc in range(HC):
                    nc.sync.dma_start(
                        xT[:, hc, r0:r1],
                        xbf[r0:r1, 128 * hc : 128 * (hc + 1)],
                        transpose=True,
                    )
            for q in range(1, 4):
                r0, r1 = q * 1024, (q + 1) * 1024
                nc.gpsimd.dma_start(xbf[r0:r1, :], x[r0:r1, :])
                for hc in range(HC):
                    nc.sync.dma_start(
                        xT[:, hc, r0:r1],
                        xbf[r0:r1, 128 * hc : 128 * (hc + 1)],
                        transpose=True,
                    )

        with (
            tc.tile_pool(name="pproj", bufs=2, space="PSUM") as pproj,
            tc.tile_pool(name="pacc", bufs=1, space="PSUM") as pacc,
            tc.tile_pool(name="psco", bufs=1, space="PSUM") as psco,
        ):
            # PE warmup while the first casts/transposes stream in
            pw = pproj.tile([128, 512], dt.float32, tag="pp")
            for _ in range(N_WARM):
                nc.tensor.matmul(
                    pw[:], wtile[:, 0:128], wtile[:], start=True, stop=True
                )
            dummy = fin.tile([128, 1], dt.float32, tag="dummy")
            nc.scalar.activation(dummy[:], wtile[:, 0:1], AF.Exp)

            emit_xt_chain()

            # --- PE work generators, emitted interleaved ---
            def proj_qv(sb):  # [Q^T; V^T] for one 512-wide s-block
                ps = pproj.tile([128, 512], dt.float32, tag="pp")
                for hc in range(HC):
                    nc.tensor.matmul(
                        ps[:],
                        wqv_sb[:, hc, :],
                        xT[:, hc, 512 * sb : 512 * (sb + 1)],
                        start=(hc == 0),
                        stop=(hc == HC - 1),
                    )
                nc.vector.tensor_copy(qt[0:64, 512 * sb : 512 * (sb + 1)], ps[0:64, :])
                nc.vector.tensor_copy(
                    vtsb[64:128, 512 * sb : 512 * (sb + 1)], ps[64:128, :]
                )

            def proj_k(sb):
                ps = pproj.tile([64, 512], dt.float32, tag="pp")
                for hc in range(HC):
                    nc.tensor.matmul(
                        ps[:],
                        wk_sb[:, hc, :],
                        xT[:, hc, 512 * sb : 512 * (sb + 1)],
                        start=(hc == 0),
                        stop=(hc == HC - 1),
                    )
                nc.vector.tensor_copy(kt[0:64, 512 * sb : 512 * (sb + 1)], ps[:])

            def vt_block(st0, st1):  # V^T -> V via PE transpose
                for st in range(st0, st1):
                    pvt = pproj.tile([128, D], dt.bfloat16, tag="pp")
                    nc.tensor.transpose(
                        pvt[:],
                        vtsb[64:128, 128 * st : 128 * (st + 1)],
                        id_bf[64:128, 64:128],
                    )
                    nc.vector.tensor_copy(vp[:, st, 0:D], pvt[:])

            acc0 = pacc.tile([D + 1, 1024], dt.float32, tag="acc0")
            acc1 = pacc.tile([D + 1, 1024], dt.float32, tag="acc1")
            acc = [acc0, acc1]

            def t_iter(t):
                for ih in range(2):
                    ps = psco.tile([128, 1024], dt.float32, tag="ps")
                    for nb in range(2):
                        nc.tensor.matmul(
                            ps[:, 512 * nb : 512 * (nb + 1)],
                            qt[0:64, 128 * t : 128 * (t + 1)],
                            kt[
                                0:64,
                                1024 * ih + 512 * nb : 1024 * ih + 512 * (nb + 1),
                            ],
                            start=True,
                            stop=True,
                        )
                    pt = ptile.tile([128, 1024], dt.bfloat16)
                    nc.scalar.activation(
                        pt[:], ps[:], AF.Exp, bias=mb_sb[:, t : t + 1], scale=0.125
                    )
                    for nb in range(2):
                        nc.tensor.matmul(
                            acc[ih][:, 512 * nb : 512 * (nb + 1)],
                            vp[:, t, :],
                            pt[:, 512 * nb : 512 * (nb + 1)],
                            start=(t == 0),
                            stop=(t == JT - 1),
                        )

            # --- emission schedule: projections chase the DMA stream, then
            # the t-loop interleaves with the remaining projections ---
            proj_qv(0)
            proj_qv(1)
            vt_block(0, 8)
            proj_qv(2)
            proj_qv(3)
            for sb in range(4):
                proj_k(sb)
            vt_block(8, 12)
            for t in range(0, 4):
                t_iter(t)
            proj_qv(4)
            vt_block(12, 16)
            for t in range(4, 8):
                t_iter(t)
            proj_qv(5)
            vt_block(16, 20)
            for t in range(8, 12):
                t_iter(t)
            proj_qv(6)
            vt_block(20, 24)
            for t in range(12, 16):
                t_iter(t)
            proj_qv(7)
            vt_block(24, 32)
            for t in range(16, 32):
                t_iter(t)

            # --- finale: transpose out'^T, normalize, store ---
            oall = fin.tile([128, 16, D], dt.float32, tag="oall")
            for ih in range(2):
                acc_sb = accs.tile([D + 1, 1024], dt.float32, tag="accs")
                nc.vector.tensor_copy(acc_sb[:], acc[ih][:])
                for k in range(8):
                    po = psco.tile([128, D + 1], dt.float32, tag="ps")
                    nc.tensor.transpose(
                        po[:],
                        acc_sb[:, 128 * k : 128 * (k + 1)],
                        id_f32[0 : D + 1, 0 : D + 1],
                    )
                    rc = fin.tile([128, 1], dt.float32, tag="rc")
                    nc.vector.reciprocal(rc[:], po[:, D : D + 1])
                    nc.vector.tensor_scalar_mul(
                        oall[:, 8 * ih + k, :], po[:, 0:D], rc[:]
                    )
            nc.sync.dma_start(out.rearrange("(k p) d -> p k d", p=128), oall[:])

    nc.compile()
    return nc


def _in_maps(x, mask, Wk, Wq, Wv):
    wqv = np.ascontiguousarray(np.concatenate([Wq.T, Wv.T], axis=1), dtype=np.float32)
    wkt = np.ascontiguousarray(Wk.T, dtype=np.float32)
    ident = np.eye(128, dtype=np.float32)
    maps = []
    for c in range(N_CORES):
        b, half = c // 2, c % 2
        i0 = half * SC
        xr = np.ascontiguousarray(np.roll(x[b], -i0, axis=0))
        mr = np.roll(mask[b], -i0)
        mbv = np.where(mr == 0, np.float32(NEG), np.float32(0.0)).astype(np.float32)
        mbt = np.ascontiguousarray(mbv.reshape(JT, 128).T)  # [128, JT], j = 128*t + p
        maps.append({"x": xr, "wqv": wqv, "wkt": wkt, "mb": mbt, "ident": ident})
    return maps


def kernel(x, mask, Wk, Wq, Wv):
    from concourse.bass_utils import run_bass_kernel_spmd

    if "nc" not in _CACHE:
        _CACHE["nc"] = _build()
    nc = _CACHE["nc"]
    maps = _in_maps(x, mask, Wk, Wq, Wv)
    br = run_bass_kernel_spmd(nc, maps, list(range(N_CORES)))
    out = np.empty((B, S, D), dtype=np.float32)
    for c in range(N_CORES):
        b, half = c // 2, c % 2
        out[b, half * SC : (half + 1) * SC, :] = br.results[c]["out"]
    return out


# revision 14
# speedup vs baseline: 1.4713x; 1.4713x over previous
"""Trainium2 Bass kernel for nn_AttentionHead (B=4, S=4096, H=1024, D=64).

Reference computation (note the unusual K-first ordering):
    K = x @ Wk.T; Q = x @ Wq.T; V = x @ Wv.T            [B,S,D]
    scores[b,i,j] = (K[b,i] . Q[b,j]) / sqrt(D)         [B,S,S]
    scores[:, :, j] = -1e12 where mask[:, j] == 0
    out = softmax(scores, axis=2) @ V                   [B,S,D]

Sharding: 8 cores = 4 batches x 2 key-row chunks of 2048. Each core gets a
batch's x ROLLED so its own key rows are always rows [0, 2048) — the SPMD
program is identical across cores. Softmax runs over the full (rolled) query
axis on every core, so rolling is correctness-neutral.

Per-core pipeline (bf16 matmuls, fp32 accumulation):
  - x is DMA-cast to bf16 per s-chunk (SWDGE, one internal DRAM tensor per
    chunk — a single tensor creates false whole-tensor WAR dependencies that
    serialize the chain) and xbar-DMA-transposed into SBUF x^T (sync HWDGE
    queue only — running the xbar on both HWDGE queues corrupts data).
  - One [Wq|Wv] stationary gives Q^T (rows 0:64) and V^T (rows 64:128) per
    512-col block; K^T separately over own 2048 rows; V^T -> V via PE
    transposes (identity matmul). V gets a ones column (softmax denominator).
  - PE warmup matmuls on junk data cover the DMA ramp so the HAM clock-gate
    sits at 8/8 when real work arrives.
  - Two passes over the query-tile loop, one per 1024-wide i-half. Pass A is
    emission-interleaved with the later projections so PE always has dense
    work chasing the DMA stream. Per tile t: scores^T = Q^T_t.T @ K^T on PE;
    exp(0.125*s + maskbias[j]) on ACT (mask folded into the per-partition
    bias; masked queries underflow to exactly 0); PE accumulates
    V'_t.T @ P^T_t into out'^T [65, 1024] — rows 0:64 numerator^T, row 64
    denominator; one PSUM bank per 512-col window so accumulation groups
    never interleave within a bank.
  - Per-pass finale: PE-transpose out'^T via identity matmul, then
    out = numerator * reciprocal(denominator) on DVE; one DMA store.
"""

import numpy as np

B, S, H, D = 4, 4096, 1024, 64
N_CORES = 8
SC = S // 2  # key rows per core
HC = H // 128  # contraction chunks
JT = S // 128  # query tiles
NEG = -30000.0
N_WARM = 64

_CACHE = {}


def _build():
    import concourse.bass as bass
    import concourse.tile as tile
    from concourse import bacc, mybir

    dt = mybir.dt
    AF = mybir.ActivationFunctionType

    nc = bacc.Bacc(
        "TRN2", target_bir_lowering=False, debug=False, num_devices=N_CORES
    )
    x = nc.dram_tensor("x", [S, H], dt.float32, kind="ExternalInput").ap()
    wqv = nc.dram_tensor("wqv", [H, 2 * D], dt.float32, kind="ExternalInput").ap()
    wkt = nc.dram_tensor("wkt", [H, D], dt.float32, kind="ExternalInput").ap()
    mb = nc.dram_tensor("mb", [128, JT], dt.float32, kind="ExternalInput").ap()
    ident = nc.dram_tensor("ident", [128, 128], dt.float32, kind="ExternalInput").ap()
    out = nc.dram_tensor("out", [SC, D], dt.float32, kind="ExternalOutput").ap()

    # one bf16 staging tensor per s-chunk (avoids false WAR serialization)
    chunks = [(0, 512), (512, 1024), (1024, 2048), (2048, 3072), (3072, 4096)]
    xbfs = [
        nc.dram_tensor(f"xbf{i}", [r1 - r0, H], dt.bfloat16).ap()
        for i, (r0, r1) in enumerate(chunks)
    ]

    with (
        tile.TileContext(nc) as tc,
        tc.tile_pool(name="persist", bufs=1) as persist,
        tc.tile_pool(name="ptile", bufs=4) as ptile,
        tc.tile_pool(name="accs", bufs=2) as accs,
        tc.tile_pool(name="fin", bufs=2) as fin,
    ):
        qt = persist.tile([128, S], dt.bfloat16)  # rows 0:64 = Q^T
        kt = persist.tile([128, SC], dt.bfloat16)  # rows 0:64 = K^T
        vtsb = persist.tile([128, S], dt.bfloat16)  # rows 64:128 = V^T
        vp = persist.tile([128, JT, D + 1], dt.bfloat16)
        mb_sb = persist.tile([128, JT], dt.float32)
        id_f32 = persist.tile([128, 128], dt.float32)
        id_bf = persist.tile([128, 128], dt.bfloat16)
        wtile = persist.tile([128, 512], dt.bfloat16)
        xT = persist.tile([128, HC, S], dt.bfloat16)
        wqv_sb = persist.tile([128, HC, 2 * D], dt.bfloat16)
        wk_sb = persist.tile([128, HC, D], dt.bfloat16)

        nc.sync.dma_start(mb_sb[:], mb[:])
        nc.sync.dma_start(id_f32[:], ident[:])
        nc.gpsimd.dma_start(id_bf[:], ident[:])
        nc.gpsimd.dma_start(wqv_sb[:], wqv.rearrange("(c p) d -> p c d", p=128))
        nc.gpsimd.dma_start(wk_sb[:], wkt.rearrange("(c p) d -> p c d", p=128))
        nc.vector.memset(vp[:, :, D], 1.0)
        nc.vector.memset(wtile[:], 0.0)

        def emit_xt_chain():
            for (r0, r1), xbf in zip(chunks, xbfs):
                nc.gpsimd.dma_start(xbf[:], x[r0:r1, :])
                for hc in range(HC):
                    nc.sync.dma_start(
                        xT[:, hc, r0:r1],
                        xbf[:, 128 * hc : 128 * (hc + 1)],
                        transpose=True,
                    )

        def make_proj_qv(pool):
            def proj_qv(sb):  # [Q^T; V^T] for one 512-wide s-block
                ps = pool.tile([128, 512], dt.float32, tag="pp")
                for hc in range(HC):
                    nc.tensor.matmul(
                        ps[:],
                        wqv_sb[:, hc, :],
                        xT[:, hc, 512 * sb : 512 * (sb + 1)],
                        start=(hc == 0),
                        stop=(hc == HC - 1),
                    )
                nc.vector.tensor_copy(qt[0:64, 512 * sb : 512 * (sb + 1)], ps[0:64, :])
                nc.vector.tensor_copy(
                    vtsb[64:128, 512 * sb : 512 * (sb + 1)], ps[64:128, :]
                )

            def proj_k(sb):
                ps = pool.tile([64, 512], dt.float32, tag="pp")
                for hc in range(HC):
                    nc.tensor.matmul(
                        ps[:],
                        wk_sb[:, hc, :],
                        xT[:, hc, 512 * sb : 512 * (sb + 1)],
                        start=(hc == 0),
                        stop=(hc == HC - 1),
                    )
                nc.vector.tensor_copy(kt[0:64, 512 * sb : 512 * (sb + 1)], ps[:])

            def vt_block(st0, st1):  # V^T -> V via PE transpose
                for st in range(st0, st1):
                    pvt = pool.tile([128, D], dt.bfloat16, tag="pp")
                    nc.tensor.transpose(
                        pvt[:],
                        vtsb[64:128, 128 * st : 128 * (st + 1)],
                        id_bf[64:128, 64:128],
                    )
                    nc.vector.tensor_copy(vp[:, st, 0:D], pvt[:])

            return proj_qv, proj_k, vt_block

        def make_t_iter(psco, acc, ih):
            def t_iter(t):
                ps = psco.tile([128, 1024], dt.float32, tag="ps")
                for nb in range(2):
                    nc.tensor.matmul(
                        ps[:, 512 * nb : 512 * (nb + 1)],
                        qt[0:64, 128 * t : 128 * (t + 1)],
                        kt[0:64, 1024 * ih + 512 * nb : 1024 * ih + 512 * (nb + 1)],
                        start=True,
                        stop=True,
                    )
                pt = ptile.tile([128, 1024], dt.bfloat16)
                nc.scalar.activation(
                    pt[:], ps[:], AF.Exp, bias=mb_sb[:, t : t + 1], scale=0.125
                )
                for nb in range(2):
                    nc.tensor.matmul(
                        acc[:, 512 * nb : 512 * (nb + 1)],
                        vp[:, t, :],
                        pt[:, 512 * nb : 512 * (nb + 1)],
                        start=(t == 0),
                        stop=(t == JT - 1),
                    )

            return t_iter

        def finale(psco, acc_sb, ih):
            for k in range(8):
                po = psco.tile([128, D + 1], dt.float32, tag="ps")
                nc.tensor.transpose(
                    po[:],
                    acc_sb[:, 128 * k : 128 * (k + 1)],
                    id_f32[0 : D + 1, 0 : D + 1],
                )
                rc = fin.tile([128, 1], dt.float32, tag="rc")
                nc.vector.reciprocal(rc[:], po[:, D : D + 1])
                nc.vector.tensor_scalar_mul(oall[:, 8 * ih + k, :], po[:, 0:D], rc[:])

        oall = fin.tile([128, 16, D], dt.float32, tag="oall")

        # ---- pass A (i-half 0), interleaved with projections ----
        with tc.tile_pool(name="paccA", bufs=1, space="PSUM") as paccA:
            accA = paccA.tile([D + 1, 1024], dt.float32, tag="acc")
            with (
                tc.tile_pool(name="pproj", bufs=2, space="PSUM") as pproj,
                tc.tile_pool(name="pscoA", bufs=2, space="PSUM") as pscoA,
            ):
                proj_qv, proj_k, vt_block = make_proj_qv(pproj)
                t_iter = make_t_iter(pscoA, accA, 0)

                pw = pproj.tile([128, 512], dt.float32, tag="pp")
                for _ in range(N_WARM):
                    nc.tensor.matmul(
                        pw[:], wtile[:, 0:128], wtile[:], start=True, stop=True
                    )
                dummy = fin.tile([128, 1], dt.float32, tag="dummy")
                nc.scalar.activation(dummy[:], wtile[:, 0:1], AF.Exp)

                emit_xt_chain()

                proj_qv(0)
                proj_qv(1)
                vt_block(0, 8)
                for sb in range(4):
                    proj_k(sb)
                for t in range(0, 4):
                    t_iter(t)
                proj_qv(2)
                vt_block(8, 12)
                for t in range(4, 8):
                    t_iter(t)
                proj_qv(3)
                vt_block(12, 16)
                for t in range(8, 12):
                    t_iter(t)
                proj_qv(4)
                vt_block(16, 20)
                for t in range(12, 16):
                    t_iter(t)
                proj_qv(5)
                vt_block(20, 24)
                for t in range(16, 20):
                    t_iter(t)
                proj_qv(6)
                vt_block(24, 28)
                for t in range(20, 24):
                    t_iter(t)
                proj_qv(7)
                vt_block(28, 32)
                for t in range(24, 32):
                    t_iter(t)
                acc_sbA = accs.tile([D + 1, 1024], dt.float32, tag="accs")
                nc.vector.tensor_copy(acc_sbA[:], accA[:])

        # ---- finale A + pass B (i-half 1) ----
        with (
            tc.tile_pool(name="pscoB", bufs=3, space="PSUM") as pscoB,
            tc.tile_pool(name="paccB", bufs=1, space="PSUM") as paccB,
        ):
            finale(pscoB, acc_sbA, 0)
            accB = paccB.tile([D + 1, 1024], dt.float32, tag="acc")
            t_iterB = make_t_iter(pscoB, accB, 1)
            for t in range(JT):
                t_iterB(t)
            acc_sbB = accs.tile([D + 1, 1024], dt.float32, tag="accs")
            nc.vector.tensor_copy(acc_sbB[:], accB[:])
            finale(pscoB, acc_sbB, 1)
        nc.sync.dma_start(out.rearrange("(k p) d -> p k d", p=128), oall[:])

    nc.compile()
    return nc


def _in_maps(x, mask, Wk, Wq, Wv):
    wqv = np.ascontiguousarray(np.concatenate([Wq.T, Wv.T], axis=1), dtype=np.float32)
    wkt = np.ascontiguousarray(Wk.T, dtype=np.float32)
    ident = np.eye(128, dtype=np.float32)
    maps = []
    for c in range(N_CORES):
        b, half = c // 2, c % 2
        i0 = half * SC
        xr = np.ascontiguousarray(np.roll(x[b], -i0, axis=0))
        mr = np.roll(mask[b], -i0)
        mbv = np.where(mr == 0, np.float32(NEG), np.float32(0.0)).astype(np.float32)
        mbt = np.ascontiguousarray(mbv.reshape(JT, 128).T)  # [128, JT], j = 128*t + p
        maps.append({"x": xr, "wqv": wqv, "wkt": wkt, "mb": mbt, "ident": ident})
    return maps


def kernel(x, mask, Wk, Wq, Wv):
    from concourse.bass_utils import run_bass_kernel_spmd

    if "nc" not in _CACHE:
        _CACHE["nc"] = _build()
    nc = _CACHE["nc"]
    maps = _in_maps(x, mask, Wk, Wq, Wv)
    br = run_bass_kernel_spmd(nc, maps, list(range(N_CORES)))
    out = np.empty((B, S, D), dtype=np.float32)
    for c in range(N_CORES):
        b, half = c // 2, c % 2
        out[b, half * SC : (half + 1) * SC, :] = br.results[c]["out"]
    return out
